# revision 4
# baseline (speedup 1.0000x reference)
"""KNN softmax loss on 8 Trainium2 NeuronCores.

Math (matching the reference):
    xn  = x / ||x||_2 (rows)                            [8192, 128]
    S   = xn @ xn.T                                     [8192, 8192]
    lse = logsumexp(S, axis=1)   (diagonal included)
    top = top-16 of each row of S with the diagonal masked
    loss = sum_i (16 * lse_i - sum(top_i))

Sharding: rows of x across the 8 cores. Each core gets np.roll(x, -c*1024)
so one identical SPMD program computes rows 0..1023 of its rotated copy
(1024 x 8192 similarity slab). Rotation only permutes each row's columns,
which both logsumexp and top-k are invariant to, and pins the self-match
of row p of row-tile rt at column rt*128+p.

Per core: normalize all of x on-chip, transpose into xnT [d=128, 8192] via
the PE, stream S row-tile blocks [128, 1024] through PSUM. The ACT engine
does fused exp+row-accumulate straight from PSUM (for lse), the DVE takes
per-512-slice top-8 candidates with the max8 instruction. A tiny global
phase (max8 / match_replace / max8 ...) extracts each row's ranks 1..24;
ranks 2..17 are the masked top-16 because the self-similarity (=1) is the
strict row max. The per-512-slice top-8 candidate set provably contains
ranks 1..8 and contains ranks 9..17 unless >8 of a row's top-17 fall in a
single 512-slice (probability ~1e-6 per problem; verified offline against
the fixed input in test.py).

The scalar partial loss of each core is summed on the host.
"""

import numpy as np

N = 8192
D = 128
NC = 8
M = N // NC          # rows per core
RT = M // 128        # 8 row-tiles per core
G = N // 1024        # 8 column groups of 1024
KK = 16

_CACHE = {}


def _build_nc():
    import concourse.mybir as mybir
    import concourse.tile as tile
    from concourse import bacc
    from concourse.masks import make_identity
    from concourse import bass_isa

    f32 = mybir.dt.float32
    AF = mybir.ActivationFunctionType
    ALU = mybir.AluOpType
    AX = mybir.AxisListType

    nc = bacc.Bacc("TRN2", target_bir_lowering=False, debug=False, num_devices=NC)
    x = nc.dram_tensor("x", [N, D], f32, kind="ExternalInput")
    out = nc.dram_tensor("out", [1, 1], f32, kind="ExternalOutput")

    with tile.TileContext(nc) as tc:
        with (
            tc.tile_pool(name="big", bufs=1) as big,
            tc.tile_pool(name="grp", bufs=8) as grp,
            tc.tile_pool(name="scr", bufs=4) as scr,
            tc.tile_pool(name="esp", bufs=3) as esp,
            tc.tile_pool(name="rtp", bufs=1) as rtp,
            tc.tile_pool(name="small", bufs=1) as small,
            tc.tile_pool(name="psA", bufs=2, space="PSUM") as psA,
            tc.tile_pool(name="psM", bufs=3, space="PSUM") as psM,
        ):
            ident = big.tile([128, 128], f32)
            make_identity(nc, ident)

            xs = big.tile([128, N], f32)    # x row-tiles side by side
            xnT = big.tile([128, N], f32)   # normalized, transposed

            # ---- load + normalize + transpose, pipelined in groups of 8 tiles
            for g in range(G):
                nc.sync.dma_start(
                    out=xs[:, g * 1024:(g + 1) * 1024].rearrange(
                        "p (t d) -> p t d", d=128
                    ),
                    in_=x[g * 1024:(g + 1) * 1024, :].rearrange(
                        "(t p) d -> p t d", p=128
                    ),
                )
            for g in range(G):
                ss = grp.tile([128, 8], f32, tag=f"ss{g}")
                for t8 in range(8):
                    t = g * 8 + t8
                    sqo = scr.tile([128, 128], f32, tag=f"sq{t8 % 4}")
                    nc.scalar.activation(
                        out=sqo,
                        in_=xs[:, t * 128:(t + 1) * 128],
                        func=AF.Square,
                        accum_out=ss[:, t8:t8 + 1],
                    )
                # rnorm = ss^-0.5 = exp(-0.5*ln(ss)); keeps ACT on one table set
                lns = grp.tile([128, 8], f32, tag=f"ln{g}")
                nc.scalar.activation(out=lns, in_=ss, func=AF.Ln)
                rn = grp.tile([128, 8], f32, tag=f"rn{g}")
                nc.scalar.activation(out=rn, in_=lns, func=AF.Exp, scale=-0.5)
                for t8 in range(8):
                    t = g * 8 + t8
                    xn = scr.tile([128, 128], f32, tag=f"xn{t8 % 4}")
                    nc.gpsimd.tensor_scalar_mul(
                        xn, xs[:, t * 128:(t + 1) * 128], rn[:, t8:t8 + 1]
                    )
                    jj = t % 4
                    if jj == 0:
                        trp = psA.tile([128, 512], f32, tag="tr")
                    nc.tensor.transpose(
                        out=trp[:, jj * 128:(jj + 1) * 128], in_=xn, identity=ident
                    )
                    if jj == 3:
                        nc.scalar.activation(
                            out=xnT[:, (t // 4) * 512:(t // 4 + 1) * 512],
                            in_=trp,
                            func=AF.Copy,
                        )

            # ---- main: stream S row-tile blocks through PSUM
            C1 = [rtp.tile([128, 128], f32, tag=f"c1_{rt}", name=f"c1_{rt}") for rt in range(RT)]
            exps = [rtp.tile([128, 8], f32, tag=f"ex{rt}", name=f"ex{rt}") for rt in range(RT)]
            for g in range(G):
                for rt in range(RT):
                    ps = psM.tile([128, 1024], f32, tag="ps")
                    lhsT = xnT[:, rt * 128:(rt + 1) * 128]
                    nc.tensor.matmul(
                        ps[:, 0:512],
                        lhsT,
                        xnT[:, g * 1024:g * 1024 + 512],
                        start=True,
                        stop=True,
                    )
                    nc.tensor.matmul(
                        ps[:, 512:1024],
                        lhsT,
                        xnT[:, g * 1024 + 512:(g + 1) * 1024],
                        start=True,
                        stop=True,
                    )
                    es = esp.tile([128, 1024], f32, tag="es")
                    nc.scalar.activation(
                        out=es,
                        in_=ps,
                        func=AF.Exp,
                        accum_out=exps[rt][:, g:g + 1],
                    )
                    nc.vector.max(
                        out=C1[rt][:, (2 * g) * 8:(2 * g) * 8 + 8], in_=ps[:, 0:512]
                    )
                    nc.vector.max(
                        out=C1[rt][:, (2 * g + 1) * 8:(2 * g + 1) * 8 + 8],
                        in_=ps[:, 512:1024],
                    )

            # ---- per-row-tile epilogue
            contribs = small.tile([128, RT], f32, tag="contribs")
            for rt in range(RT):
                est = small.tile([128, 1], f32, tag=f"est{rt}")
                nc.vector.tensor_reduce(
                    out=est, in_=exps[rt], op=ALU.add, axis=AX.X
                )
                lse = small.tile([128, 1], f32, tag=f"lse{rt}")
                nc.scalar.activation(out=lse, in_=est, func=AF.Ln)

                T = rtp.tile([128, 24], f32, tag=f"T{rt}")
                nc.vector.max(out=T[:, 0:8], in_=C1[rt])
                c1b = scr.tile([128, 128], f32, tag=f"cb{rt % 2}")
                nc.vector.match_replace(
                    out=c1b, in_to_replace=T[:, 0:8], in_values=C1[rt],
                    imm_value=-1e30,
                )
                nc.vector.max(out=T[:, 8:16], in_=c1b)
                c1c = scr.tile([128, 128], f32, tag=f"cc{rt % 2}")
                nc.vector.match_replace(
                    out=c1c, in_to_replace=T[:, 8:16], in_values=c1b,
                    imm_value=-1e30,
                )
                nc.vector.max(out=T[:, 16:24], in_=c1c)
                stp = small.tile([128, 1], f32, tag=f"st{rt}")
                nc.vector.tensor_reduce(
                    out=stp, in_=T[:, 1:17], op=ALU.add, axis=AX.X
                )
                nc.vector.scalar_tensor_tensor(
                    out=contribs[:, rt:rt + 1],
                    in0=lse,
                    scalar=16.0,
                    in1=stp,
                    op0=ALU.mult,
                    op1=ALU.subtract,
                )

            acc = small.tile([128, 1], f32, tag="acc")
            nc.vector.tensor_reduce(out=acc, in_=contribs, op=ALU.add, axis=AX.X)
            accr = small.tile([128, 1], f32, tag="accr")
            nc.gpsimd.partition_all_reduce(
                accr, acc, channels=128, reduce_op=bass_isa.ReduceOp.add
            )
            nc.sync.dma_start(out=out[:, :], in_=accr[0:1, :])

    nc.compile()
    return nc


def get_nc():
    if "nc" not in _CACHE:
        _CACHE["nc"] = _build_nc()
    return _CACHE["nc"]


def kernel(x, k):
    from concourse.bass_utils import run_bass_kernel_spmd

    x = np.ascontiguousarray(np.asarray(x, dtype=np.float32))
    assert x.shape == (N, D)
    assert int(k) == KK

    nc = get_nc()
    in_maps = [{"x": np.roll(x, -c * M, axis=0)} for c in range(NC)]
    res = run_bass_kernel_spmd(nc, in_maps, core_ids=list(range(NC)))
    loss = sum(float(r["out"][0, 0]) for r in res.results)
    return np.float32(loss)
